# revision 27
# baseline (speedup 1.0000x reference)
"""Trainium2 Bass kernel for nn_Encoders (2-layer shared-weight transformer encoder).

Sharding (v4): 8 cores; pair (2b, 2b+1) handles batch b.  Within a pair the
split is along the attention *output* token axis j (the reference's unusual
attention contracts over queries i: out[j,d] = sum_i attn[i,j] v[i,d]):

  - each core computes q and v for ALL tokens/heads (small duplication),
    k only for its own j-half,
  - E = exp(qk^T/8) * m01 for its own j columns, all heads (m01 = 1-mask,
    multiplicative 0/1 mask applied on the vector engine -- exactly equal
    to the additive -1e9 mask after exp, and removes 128 PE matmuls),
  - attention output, out-projection, residual+LN1, full-DFF FFN,
    residual+LN2 for its own j-half only -- NO partial-sum collectives.

Cross-core data: the joint-softmax denominator Z (per head-pair) is summed
with tiny 8-byte AllReduces launched as each pair finishes (latency hidden
under the remaining attention), and the layer-0 output h is exchanged with
four 256KB AllReduces launched per k-chunk as LN2's normalize produces them
(pipelined): peer = (own+peer) - own, reconstructed on-chip (~1 ulp).  Each
core keeps h in LOCAL token order (own half first; the host permutes x and
mask rows to match) so the SPMD program needs no role branches.  The final
layer outputs each core's own half directly (no exchange).

v4 scheduling: the whole layer is emitted as one interleaved instruction
stream so the PE never idles (p-state ramp keeps it at 2.4 GHz): attention
(hp, it, hr) steps are software-pipelined (lookahead 2) with the exp (scalar)
and mask-mult (DVE 4x mode) off the tensor queue, and the remaining
projection matmuls are drizzled one-per-step into the attention stream.
Layer 1 runs own-token attention for pairs 0-1 between the own-projections
and the peer-projections so the h-exchange latency is hidden.

Everything stays in transposed layout [feature, token]; LN stats via all-ones
matmul (broadcast sums).  Matmul operands are bf16 (full-rate PE + fast weight
loads); the residual stream, LN statistics and softmax sums stay f32/f32r.
A ones-column appended to v (via host-built wva/bva) makes the attnV matmul
emit per-column E sums for free -> Z partials without activation accumulators.
"""

import sys

sys.path.insert(0, "/opt/trn_rl_repo")

import numpy as np
import ml_dtypes

import concourse.bass as bass
import concourse.mybir as mybir
import concourse.tile as tile
from concourse import bacc
from concourse.bass_utils import run_bass_kernel_spmd

F32 = mybir.dt.float32
F32R = mybir.dt.float32r
BF16 = mybir.dt.bfloat16
AF = mybir.ActivationFunctionType
OP = mybir.AluOpType
AX = mybir.AxisListType

B, S, D, H, DFF = 4, 1024, 512, 8, 2048
DEPTH = D // H  # 64
NEG = -1.0e9
EPS = 1e-9
N_CORES = 8
GROUPS = [[0, 1], [2, 3], [4, 5], [6, 7]]

SJ = S // 2       # own token half: 512
KD = D // 128     # 4 k-tiles over D
IT = S // 128     # 8 i tiles
KF = DFF // 128   # 16 dff tiles
VA = 2 * (4 * 65)  # v augmented with a ones column per head: 2 halves x 260


def _rd(ap):
    return ap.bitcast(F32)


def build():
    nc = bacc.Bacc("TRN2", target_bir_lowering=False, debug=False,
                   num_devices=N_CORES)

    def din(name, shape, dt=F32):
        return nc.dram_tensor(name, shape, dt, kind="ExternalInput").ap()

    xT = din("xT", [D, SJ], F32R)
    xb = din("xb", [D, S], BF16)
    maskp = din("maskp", [S, SJ], BF16)         # m01 = 1 - mask[:, own j]
    wq = din("wq", [D, D], BF16)
    wk = din("wk", [D, D], BF16)
    wva = din("wva", [D, VA], BF16)             # v weights with ones-cols
    wo = din("wo", [D, D], BF16)
    w1 = din("w1", [D, DFF], BF16)
    w2 = din("w2", [DFF, D], BF16)
    bq = din("bq", [128, KD])
    bk = din("bk", [128, KD])
    bva = din("bva", [128, VA], BF16)
    bo = din("bo", [128, KD])
    b1 = din("b1", [128, KF])
    b2 = din("b2", [128, KD])
    g1 = din("g1", [128, KD])
    be1 = din("be1", [128, KD])
    g2 = din("g2", [128, KD])
    be2 = din("be2", [128, KD])
    id8 = din("id8", [8, 8])
    selp = din("selp", [8, 128], F32R)
    hout = nc.dram_tensor("hout", [D, SJ], F32, kind="ExternalOutput").ap()
    DBG = bool(__import__("os").environ.get("KDBG"))
    dbg = {}
    if DBG:
        dbg["q"] = nc.dram_tensor("dbg_q", [128, KD, S], BF16, kind="ExternalOutput").ap()
        dbg["k"] = nc.dram_tensor("dbg_k", [128, KD, SJ], BF16, kind="ExternalOutput").ap()
        dbg["v"] = nc.dram_tensor("dbg_v", [128, IT, 2, 260], BF16, kind="ExternalOutput").ap()
        dbg["e"] = nc.dram_tensor("dbg_e", [128, SJ], BF16, kind="ExternalOutput").ap()
        dbg["o"] = nc.dram_tensor("dbg_o", [128, KD, SJ], BF16, kind="ExternalOutput").ap()
        dbg["z1"] = nc.dram_tensor("dbg_z1", [128, KD, SJ], F32, kind="ExternalOutput").ap()
        dbg["h1"] = nc.dram_tensor("dbg_h1", [128, KD, SJ], F32, kind="ExternalOutput").ap()
        dbg["zp"] = nc.dram_tensor("dbg_zp", [65, 8], F32, kind="ExternalOutput").ap()
        dbg["zc"] = nc.dram_tensor("dbg_zc", [2, KD], F32, kind="ExternalOutput").ap()
        dbg["zv"] = nc.dram_tensor("dbg_zv", [128, KD], F32, kind="ExternalOutput").ap()

    with tile.TileContext(nc) as tc:
        with (
            tc.tile_pool(name="const", bufs=1) as const,
            tc.tile_pool(name="state", bufs=1) as state,
            tc.tile_pool(name="scr", bufs=1) as scr,
            tc.tile_pool(name="psum", bufs=2, space="PSUM") as psum,
            tc.tile_pool(name="dram", bufs=1, space="DRAM") as dram,
        ):
            def loadw(name, src, kt, m, dt=BF16):
                t = const.tile([128, kt, m], dt, name=name, tag=name)
                nc.sync.dma_start(out=t, in_=src.rearrange("(k p) m -> p k m", p=128))
                return t

            def loadsm(name, src, dt=F32):
                t = const.tile(list(src.shape), dt, name=name, tag=name)
                nc.sync.dma_start(out=t, in_=src)
                return t

            # ---- loads, ordered so the first projections start ASAP ----
            hb = state.tile([128, KD, S], BF16, name="hb0", tag="hb", bufs=2)
            nc.sync.dma_start(out=hb[:, :, 0:SJ],
                              in_=xb[:, 0:SJ].rearrange("(k p) s -> p k s", p=128))
            wk_sb = const.tile([128, KD, D], BF16, name="wk_sb", tag="wk_sb")
            wq_sb = const.tile([128, KD, D], BF16, name="wq_sb", tag="wq_sb")
            nc.sync.dma_start(out=wk_sb[:, :, 0:128],
                              in_=wk[:, 0:128].rearrange("(k p) m -> p k m", p=128))
            nc.sync.dma_start(out=wq_sb[:, :, 0:128],
                              in_=wq[:, 0:128].rearrange("(k p) m -> p k m", p=128))
            nc.sync.dma_start(out=wk_sb[:, :, 128:D],
                              in_=wk[:, 128:D].rearrange("(k p) m -> p k m", p=128))
            nc.sync.dma_start(out=wq_sb[:, :, 128:D],
                              in_=wq[:, 128:D].rearrange("(k p) m -> p k m", p=128))
            nc.sync.dma_start(out=hb[:, :, SJ:S],
                              in_=xb[:, SJ:S].rearrange("(k p) s -> p k s", p=128))
            wva_sb = loadw("wva_sb", wva, KD, VA)
            bk_sb = loadsm("bk_sb", bk)
            bq_sb = loadsm("bq_sb", bq)
            bva_sb = loadsm("bva_sb", bva, BF16)
            m01_sb = const.tile([128, IT, SJ], BF16, name="m01_sb", tag="m01_sb")
            nc.sync.dma_start(out=m01_sb, in_=maskp.rearrange("(i p) j -> p i j", p=128))
            h_cur = state.tile([128, KD, SJ], F32R, name="h0", tag="h", bufs=2)
            nc.sync.dma_start(out=h_cur, in_=xT.rearrange("(k p) s -> p k s", p=128))
            ones_sq = const.tile([128, 128], F32R, name="ones_sq", tag="ones_sq")
            nc.vector.memset(_rd(ones_sq), 1.0)
            ones_bf = const.tile([1, 128], BF16, name="ones_bf", tag="ones_bf")
            nc.vector.memset(ones_bf, 1.0)
            wo_sb = loadw("wo_sb", wo, KD, D)
            bo_sb = loadsm("bo_sb", bo)
            w1_sb = loadw("w1_sb", w1, KD, DFF)
            b1_sb = loadsm("b1_sb", b1)
            g1_sb = loadsm("g1_sb", g1)
            be1_sb = loadsm("be1_sb", be1)
            w2_sb = loadw("w2_sb", w2, KF, D)
            b2_sb = loadsm("b2_sb", b2)
            g2_sb = loadsm("g2_sb", g2)
            be2_sb = loadsm("be2_sb", be2)
            zeros_sb = const.tile([128, SJ], F32, name="zeros_sb", tag="zeros_sb")
            nc.vector.memset(zeros_sb, 0.0)
            eps_sb = const.tile([128, 1], F32, name="eps_sb", tag="eps_sb")
            nc.vector.memset(eps_sb, EPS)
            id8_sb = loadsm("id8_sb", id8)
            selp_sb = loadsm("selp_sb", selp, F32R)

            def layernorm(z, g_sb, be_sb, out_name, out_tile, out_bf=None):
                """z: f32r [128, KD, SJ]; writes normalized f32r into out_tile.
                If out_bf is given, also writes a bf16 copy per k (fused into
                the emission so downstream can start at k granularity)."""
                s1 = psum.tile([128, SJ], F32, name=f"s1_{out_name}", tag="lp", bufs=4)
                s2 = psum.tile([128, SJ], F32, name=f"s2_{out_name}", tag="lp", bufs=4)
                for k in range(KD):
                    sqc = scr.tile([128, SJ], F32R, name=f"sq_{out_name}_{k}",
                                   tag="e", bufs=8)
                    nc.scalar.activation(out=sqc, in_=_rd(z[:, k, :]), func=AF.Square)
                    nc.tensor.matmul(s1, lhsT=ones_sq, rhs=z[:, k, :],
                                     start=(k == 0), stop=(k == KD - 1))
                    nc.tensor.matmul(s2, lhsT=ones_sq, rhs=sqc,
                                     start=(k == 0), stop=(k == KD - 1))
                nmean = scr.tile([128, SJ], F32, name=f"nmean_{out_name}", tag="mean", bufs=1)
                rstd = scr.tile([128, SJ], F32, name=f"rstd_{out_name}", tag="rstd", bufs=1)
                nc.vector.tensor_scalar(out=nmean, in0=s1, scalar1=-1.0 / D,
                                        scalar2=None, op0=OP.mult)
                msq = scr.tile([128, SJ], F32, name=f"msq_{out_name}", tag="e", bufs=8)
                nc.scalar.activation(out=msq, in_=s1, func=AF.Square, scale=1.0 / D)
                var = scr.tile([128, SJ], F32, name=f"var_{out_name}", tag="e", bufs=8)
                nc.vector.scalar_tensor_tensor(out=var, in0=s2, scalar=1.0 / D,
                                               in1=msq, op0=OP.mult, op1=OP.subtract)
                nc.scalar.activation(out=var, in_=var, func=AF.Sqrt, bias=eps_sb[:, 0:1])
                nc.vector.reciprocal_approx_fast(out=rstd, in_=var)
                for k in range(KD):
                    t = scr.tile([128, SJ], F32, name=f"t_{out_name}_{k}",
                                 tag="e", bufs=8)
                    nc.vector.tensor_tensor(out=t, in0=_rd(z[:, k, :]),
                                            in1=nmean, op=OP.add)
                    nc.vector.tensor_tensor(out=t, in0=t, in1=rstd,
                                            op=OP.mult)
                    nc.vector.tensor_scalar(out=out_tile[:, k, :], in0=t,
                                            scalar1=g_sb[:, k:k + 1],
                                            scalar2=be_sb[:, k:k + 1],
                                            op0=OP.mult, op1=OP.add)
                    if out_bf is not None:
                        nc.vector.tensor_copy(out=out_bf[:, k, :],
                                              in_=_rd(out_tile[:, k, :]))

            # --- single-step emitters (closures bind current layer tiles) ---
            def mk_proj_steps(ly, qT, kT, v_sb, hb_t, own_only):
                """Return a list of thunks, each emitting ONE tensor matmul
                (plus its trailing psum->sbuf copy when it completes a tile).
                Order: k m, q m (own[, peer]) for m=0,1; v hf0; then the rest.
                own_only: emit only own-token-half q and v (layer 1 phase A)."""
                steps = []
                state_ps = {}

                def kq_mm(w_sb, b_sb, dst, m, k, cs, ce, tag):
                    def f():
                        key = (tag, m, cs)
                        if k == 0:
                            state_ps[key] = psum.tile(
                                [128, ce - cs], F32, name=f"{tag}_{ly}_{m}_{cs}",
                                tag="lp", bufs=4)
                        nc.tensor.matmul(state_ps[key],
                                         lhsT=w_sb[:, k, m * 128:(m + 1) * 128],
                                         rhs=hb_t[:, k, cs:ce],
                                         start=(k == 0), stop=(k == KD - 1))
                        if k == KD - 1:
                            nc.vector.tensor_scalar(
                                out=dst[:, m, cs:ce], in0=state_ps[key],
                                scalar1=b_sb[:, m:m + 1], scalar2=None, op0=OP.add)
                    return f

                def v_mm(it, hf, k):
                    def f():
                        key = ("v", it, hf)
                        if k == 0:
                            state_ps[key] = psum.tile(
                                [128, 260], F32, name=f"v_ps_{ly}_{it}_{hf}",
                                tag="lp", bufs=4)
                        v_ps = state_ps[key]
                        nc.tensor.matmul(
                            v_ps, lhsT=hb_t[:, k, it * 128:(it + 1) * 128],
                            rhs=wva_sb[:, k, hf * 260:(hf + 1) * 260],
                            start=(k == 0), stop=(k == KD - 1))
                        if k == KD - 1:
                            nc.vector.tensor_tensor(
                                out=v_sb[:, it, hf, :], in0=v_ps,
                                in1=bva_sb[:, hf * 260:(hf + 1) * 260],
                                op=OP.add)
                    return f

                its = list(range(IT // 2)) if own_only else list(range(IT))
                # all k + q first, then v hf0, then v hf1.  The split point
                # returned separates safely-drizzlable steps (consumers are
                # >= ~45 PE instructions away; LDWEIGHTS pull-ahead reads
                # lhsT tiles early, so near writes race) from the rest.
                for m in range(KD):
                    for k in range(KD):
                        steps.append(kq_mm(wk_sb, bk_sb, kT, m, k, 0, SJ, "k"))
                    for k in range(KD):
                        steps.append(kq_mm(wq_sb, bq_sb, qT, m, k, 0, SJ, "qo"))
                    if m == 0:
                        pass
                if not own_only:
                    for m in range(KD):
                        for k in range(KD):
                            steps.append(kq_mm(wq_sb, bq_sb, qT, m, k, SJ, S, "qp"))
                for it in its:
                    for k in range(KD):
                        steps.append(v_mm(it, 0, k))
                for it in its[:len(its) // 2]:
                    for k in range(KD):
                        steps.append(v_mm(it, 1, k))
                pre_n = len(steps)
                for it in its[len(its) // 2:]:
                    for k in range(KD):
                        steps.append(v_mm(it, 1, k))
                return steps, pre_n

            def mk_peer_steps(ly, qT, v_sb, hb_t):
                """Peer-half q and v projections for layer 1 (after exchange)."""
                steps = []
                state_ps = {}

                def q_mm(m, k):
                    def f():
                        key = ("qp", m)
                        if k == 0:
                            state_ps[key] = psum.tile(
                                [128, SJ], F32, name=f"qp_ps_{ly}_{m}", tag="lp", bufs=4)
                        nc.tensor.matmul(state_ps[key],
                                         lhsT=wq_sb[:, k, m * 128:(m + 1) * 128],
                                         rhs=hb_t[:, k, SJ:S],
                                         start=(k == 0), stop=(k == KD - 1))
                        if k == KD - 1:
                            nc.vector.tensor_scalar(
                                out=qT[:, m, SJ:S], in0=state_ps[key],
                                scalar1=bq_sb[:, m:m + 1], scalar2=None, op0=OP.add)
                    return f

                def v_mm(it, hf, k):
                    def f():
                        key = ("v", it, hf)
                        if k == 0:
                            state_ps[key] = psum.tile(
                                [128, 260], F32, name=f"vp_ps_{ly}_{it}_{hf}",
                                tag="lp", bufs=4)
                        v_ps = state_ps[key]
                        nc.tensor.matmul(
                            v_ps, lhsT=hb_t[:, k, it * 128:(it + 1) * 128],
                            rhs=wva_sb[:, k, hf * 260:(hf + 1) * 260],
                            start=(k == 0), stop=(k == KD - 1))
                        if k == KD - 1:
                            nc.vector.tensor_tensor(
                                out=v_sb[:, it, hf, :], in0=v_ps,
                                in1=bva_sb[:, hf * 260:(hf + 1) * 260],
                                op=OP.add)
                    return f

                for m in range(KD):
                    for k in range(KD):
                        steps.append(q_mm(m, k))
                for hf in (0, 1):
                    for it in range(IT // 2, IT):
                        for k in range(KD):
                            steps.append(v_mm(it, hf, k))
                return steps

            def mk_attn(ly, qT, kT, v_sb, o_ps, o_started):
                """Per-(hp,it,hr)-step attention emitters, software-pipelined
                by the caller.  qk goes to PSUM tag 'lp'; exp (scalar) and
                m01-mult (DVE 4x) bridge to the av matmul."""
                lps = {}

                def qk(hp, it, hr):
                    pb = 64 * hr
                    l_ps = psum.tile([128, SJ], F32, name=f"l_{ly}_{hp}_{it}_{hr}",
                                     tag="lp", bufs=4)
                    nc.tensor.matmul(l_ps,
                                     lhsT=qT[pb:pb + 64, hp, it * 128:(it + 1) * 128],
                                     rhs=kT[pb:pb + 64, hp, :],
                                     start=True, stop=True)
                    et = scr.tile([128, SJ], BF16, name=f"et_{ly}_{hp}_{it}_{hr}",
                                  tag="e", bufs=8)
                    nc.scalar.activation(out=et, in_=l_ps, func=AF.Exp, scale=0.125)
                    e2 = scr.tile([128, SJ], BF16, name=f"e2_{ly}_{hp}_{it}_{hr}",
                                  tag="e2", bufs=6)
                    nc.vector.tensor_tensor(out=e2, in0=et, in1=m01_sb[:, it, :],
                                            op=OP.mult)
                    if DBG and ly == 0 and (hp, it, hr) == (0, 0, 0):
                        nc.sync.dma_start(out=dbg["e"], in_=e2)
                    lps[(hp, it, hr)] = e2

                def av(hp, it, hr, last):
                    h_abs = 2 * hp + hr
                    first = not o_started[2 * hp + hr]
                    o_started[2 * hp + hr] = True
                    nc.tensor.matmul(
                        o_ps[hp][hr],
                        lhsT=v_sb[:, it, h_abs // 4,
                                  65 * (h_abs % 4):65 * (h_abs % 4) + 65],
                        rhs=lps.pop((hp, it, hr)),
                        start=first, stop=last)
                return qk, av

            def zred(hp, o_ps, zparts):
                for hr in range(2):
                    h_abs = 2 * hp + hr
                    nc.vector.reduce_sum(out=zparts[64:65, h_abs:h_abs + 1],
                                         in_=o_ps[hp][hr][64:65, :], axis=AX.X)

            def zcc_launch(ly, hp, zparts):
                ci = dram.tile([1, 2], F32, name=f"ccz_in_{ly}_{hp}",
                               tag=f"ccz_in_{ly}_{hp}")
                co = dram.tile([1, 2], F32, name=f"ccz_out_{ly}_{hp}",
                               tag=f"ccz_out_{ly}_{hp}")
                nc.sync.dma_start(out=ci, in_=zparts[64:65, 2 * hp:2 * hp + 2])
                nc.gpsimd.collective_compute("AllReduce", OP.add, replica_groups=GROUPS,
                                             ins=[ci.opt()], outs=[co.opt()])
                return co

            def outT_copy(ly, hp, o_ps, outT):
                for hr in range(2):
                    nc.vector.tensor_copy(out=outT[hp][64 * hr:64 * hr + 64, :],
                                          in_=o_ps[hp][hr][0:64, :])

            def zchain_c(ly, hp, co, outT, ap_ps, z1t, h_cur):
                """1/Z for pair hp, scale outT pair, accumulate its out-proj part"""
                z2c = scr.tile([2, 1], F32, name=f"z2c_{ly}_{hp}", tag="z2c", bufs=2)
                nc.sync.dma_start(out=z2c, in_=bass.AP(tensor=co.tensor,
                                                       offset=co.offset,
                                                       ap=[[1, 2], [1, 1]]))
                z2i = scr.tile([2, 1], F32, name=f"z2i_{ly}_{hp}", tag="z2i", bufs=2)
                nc.vector.reciprocal(out=z2i, in_=z2c)
                dg2 = scr.tile([2, 2], F32R, name=f"dg2_{ly}_{hp}", tag="dg2", bufs=2)
                nc.vector.tensor_scalar(out=dg2, in0=id8_sb[0:2, 0:2], scalar1=z2i,
                                        scalar2=None, op0=OP.mult)
                zps = psum.tile([128, 2], F32, name=f"zps_{ly}_{hp}", tag="lp", bufs=4)
                nc.tensor.matmul(zps, lhsT=selp_sb[0:2, :], rhs=dg2,
                                 start=True, stop=True)
                zinv = scr.tile([128, 1], F32, name=f"zinv_{ly}_{hp}", tag="zinv", bufs=2)
                nc.vector.reduce_sum(out=zinv, in_=zps, axis=AX.X)
                if DBG and ly == 0:
                    nc.sync.dma_start(out=dbg["zc"][:, hp:hp+1], in_=z2c)
                    nc.sync.dma_start(out=dbg["zv"][:, hp:hp+1], in_=zinv)
                nc.vector.tensor_scalar(out=outT[hp], in0=outT[hp],
                                        scalar1=zinv, scalar2=None, op0=OP.mult)
                for dt_ in range(KD):
                    nc.tensor.matmul(ap_ps[dt_],
                                     lhsT=wo_sb[:, hp, dt_ * 128:(dt_ + 1) * 128],
                                     rhs=outT[hp],
                                     start=(hp == 0), stop=(hp == KD - 1))
                    if hp == KD - 1:
                        nc.vector.scalar_tensor_tensor(
                            out=z1t[:, dt_, :], in0=ap_ps[dt_],
                            scalar=bo_sb[:, dt_:dt_ + 1],
                            in1=_rd(h_cur[:, dt_, :]), op0=OP.add, op1=OP.add)

            # ================= main layer loop =================
            pend = None  # deferred boundary work (set at end of layer 0)
            for ly in range(2):
                qT = state.tile([128, KD, S], BF16, name=f"qT_{ly}", tag="qT", bufs=1)
                kT = state.tile([128, KD, SJ], BF16, name=f"kT_{ly}", tag="kT", bufs=1)
                v_sb = state.tile([128, IT, 2, 260], BF16, name=f"v_{ly}", tag="v",
                                  bufs=1)
                o_ps = [[psum.tile([65, SJ], F32, name=f"o_{ly}_{hp}_{hr}", tag="ob", bufs=4)
                         for hr in range(2)] for hp in range(KD)]
                o_started = [False] * 8
                outT = [state.tile([128, SJ], BF16, name=f"outT_{ly}_{p}", tag="outT",
                                   bufs=KD) for p in range(KD)]
                zparts = scr.tile([65, 8], F32, name=f"zp_{ly}", tag="zp", bufs=1)
                qk, av = mk_attn(ly, qT, kT, v_sb, o_ps, o_started)

                # attention step order: layer 0 plain hp-major; layer 1 runs
                # pairs 0-1 own-half first (fills the h-exchange window), then
                # peer projections, then the rest.
                if ly == 0:
                    proj, pre_n = mk_proj_steps(ly, qT, kT, v_sb, hb, False)
                    pre_n = len(proj)
                    sched = [("p", i) for i in range(pre_n)]
                    asteps = [(hp, it, hr) for hp in range(KD)
                              for it in range(IT) for hr in range(2)]
                    pi = pre_n
                    for si, a in enumerate(asteps):
                        sched.append(("a", a))
                        if pi < len(proj):  # drizzle 1 proj mm per attn step
                            sched.append(("p", pi)); pi += 1
                    while pi < len(proj):
                        sched.append(("p", pi)); pi += 1
                else:
                    proj, _ = mk_proj_steps(ly, qT, kT, v_sb, hb, True)
                    peer = mk_peer_steps(ly, qT, v_sb, hb)
                    own01 = [(hp, it, hr) for hp in (0, 1)
                             for it in range(IT // 2) for hr in range(2)]
                    rest = ([(hp, it, hr) for hp in (0, 1)
                             for it in range(IT // 2, IT) for hr in range(2)]
                            + [(hp, it, hr) for hp in (2, 3)
                               for it in range(IT) for hr in range(2)])
                    sched = [("p", i) for i in range(len(proj))]
                    sched += [("a", a) for a in own01]
                    sched += [("x", i) for i in range(len(peer))]
                    sched += [("a", a) for a in rest]

                # --- emit the schedule with attention lookahead 3 ---
                LA = 3
                pending = []   # attention steps awaiting their av
                counts = {}
                a_list = [a for kind, a in sched if kind == "a"]
                for hp, it, hr in a_list:
                    counts[(hp, hr)] = counts.get((hp, hr), 0) + 1
                seen = {}
                done_pairs = set()
                av_done = {}

                def emit_av(a):
                    hp, it, hr = a
                    seen[(hp, hr)] = seen.get((hp, hr), 0) + 1
                    last = seen[(hp, hr)] == counts[(hp, hr)]
                    av(hp, it, hr, last)
                    if last:
                        av_done[(hp, hr)] = True
                        if av_done.get((hp, 0)) and av_done.get((hp, 1)):
                            done_pairs.add(hp)
                            zred(hp, o_ps, zparts)
                            cos.append((hp, zcc_launch(ly, hp, zparts)))
                            outT_copy(ly, hp, o_ps, outT)

                cos = []
                for kind, a in sched:
                    if kind == "p":
                        proj[a]()
                    elif kind == "x":
                        if a == 0 and pend is not None:
                            # unmix the h-exchange before peer projections
                            ccsum = state.tile([128, KD, SJ], F32,
                                               name="ccsum", tag="ccs", bufs=1)
                            nc.sync.dma_start(out=ccsum, in_=pend["ccs_out"])
                            for k in range(KD):
                                nc.vector.tensor_tensor(
                                    out=pend["hb"][:, k, SJ:S],
                                    in0=ccsum[:, k, :],
                                    in1=_rd(pend["h"][:, k, :]),
                                    op=OP.subtract)
                        peer[a]()
                    else:
                        qk(*a)
                        pending.append(a)
                        if len(pending) > LA:
                            emit_av(pending.pop(0))
                while pending:
                    emit_av(pending.pop(0))

                if DBG and ly == 0:
                    nc.sync.dma_start(out=dbg["zp"], in_=zparts)
                    nc.sync.dma_start(out=dbg["q"], in_=qT)
                    nc.sync.dma_start(out=dbg["k"], in_=kT)
                    nc.sync.dma_start(out=dbg["v"], in_=v_sb)
                # ---- Z chains + out-projection accumulation ----
                ap_ps = [psum.tile([128, SJ], F32, name=f"ap_ps_{ly}_{d}", tag="ob", bufs=4)
                         for d in range(KD)]
                z1 = state.tile([128, KD, SJ], F32R, name=f"z1_{ly}", tag="qz", bufs=1)
                for hp, co in cos:
                    zchain_c(ly, hp, co, outT, ap_ps, z1, h_cur)
                if DBG and ly == 0:
                    for p in range(KD):
                        nc.sync.dma_start(out=dbg["o"][:, p, :], in_=outT[p])
                    nc.sync.dma_start(out=dbg["z1"], in_=_rd(z1))

                # ---- LN1 (with fused bf16 copy) ----
                h1 = state.tile([128, KD, SJ], F32R, name=f"h1_{ly}", tag="h1", bufs=1)
                h1b = state.tile([128, KD, SJ], BF16, name=f"h1b_{ly}", tag="h1b", bufs=1)
                layernorm(z1, g1_sb, be1_sb, f"h1_{ly}", h1, out_bf=h1b)
                if DBG and ly == 0:
                    nc.sync.dma_start(out=dbg["h1"], in_=_rd(h1))

                # ---- FFN (full DFF, own j-half), software-pipelined ----
                z2 = state.tile([128, KD, SJ], F32R, name=f"z2_{ly}", tag="qz", bufs=1)
                g_ps = [psum.tile([128, SJ], F32, name=f"g_ps_{ly}_{d}", tag="ob", bufs=4)
                        for d in range(KD)]
                frs = {}

                def ffn_f(ft):
                    f_ps = psum.tile([128, SJ], F32, name=f"f_ps_{ly}_{ft}", tag="lp", bufs=4)
                    for k in range(KD):
                        nc.tensor.matmul(f_ps, lhsT=w1_sb[:, k, ft * 128:(ft + 1) * 128],
                                         rhs=h1b[:, k, :],
                                         start=(k == 0), stop=(k == KD - 1))
                    fr = scr.tile([128, SJ], BF16, name=f"fr_{ly}_{ft}", tag="fr", bufs=4)
                    if ft % 2 == 0:
                        nc.vector.scalar_tensor_tensor(out=fr, in0=f_ps,
                                                       scalar=b1_sb[:, ft:ft + 1],
                                                       in1=zeros_sb, op0=OP.add,
                                                       op1=OP.max)
                    else:
                        nc.scalar.activation(out=fr, in_=f_ps, func=AF.Relu,
                                             bias=b1_sb[:, ft:ft + 1])
                    frs[ft] = fr

                def ffn_g(ft):
                    fr = frs.pop(ft)
                    for d in range(KD):
                        nc.tensor.matmul(g_ps[d],
                                         lhsT=w2_sb[:, ft, d * 128:(d + 1) * 128],
                                         rhs=fr,
                                         start=(ft == 0), stop=(ft == KF - 1))
                        if ft == KF - 1:
                            nc.vector.scalar_tensor_tensor(
                                out=z2[:, d, :], in0=g_ps[d],
                                scalar=b2_sb[:, d:d + 1],
                                in1=_rd(h1[:, d, :]), op0=OP.add, op1=OP.add)

                for ft in range(KF):
                    ffn_f(ft)
                    if ft >= 1:
                        ffn_g(ft - 1)
                ffn_g(KF - 1)

                # ---- LN2 -> chunked h-exchange (or final output) ----
                if ly == 0:
                    h_next = state.tile([128, KD, SJ], F32R, name=f"h_{ly + 1}",
                                        tag="h", bufs=2)
                    hb_next = state.tile([128, KD, S], BF16, name=f"hb_{ly + 1}",
                                         tag="hb", bufs=2)
                    layernorm(z2, g2_sb, be2_sb, f"hs_{ly}", h_next)
                    for k in range(KD):
                        nc.vector.tensor_copy(out=hb_next[:, k, 0:SJ],
                                              in_=_rd(h_next[:, k, :]))
                    ci = dram.tile([128, KD, SJ], F32, name="ccs_in", tag="ccs_in")
                    co = dram.tile([128, KD, SJ], F32, name="ccs_out", tag="ccs_out")
                    nc.sync.dma_start(out=ci, in_=_rd(h_next))
                    nc.gpsimd.collective_compute(
                        "AllReduce", OP.add, replica_groups=GROUPS,
                        ins=[ci.opt()], outs=[co.opt()])
                    pend = {"ccs_out": co, "h": h_next, "hb": hb_next}
                    h_cur = h_next
                    hb = hb_next
                else:
                    hstage = state.tile([128, KD, SJ], F32R, name=f"hs_{ly}",
                                        tag="hst", bufs=1)
                    layernorm(z2, g2_sb, be2_sb, f"hs_{ly}", hstage)
                    houtr = hout.rearrange("(k p) s -> p k s", p=128)
                    for k in range(KD):
                        nc.sync.dma_start(out=houtr[:, k, :],
                                          in_=_rd(hstage[:, k, :]))

    nc.compile()
    return nc


_CACHE = {}


def _prep_inputs(x, mask, Wq, bq, Wk, bk, Wv, bv, Wo, bo, W1, b1, W2, b2,
                 g1, be1, g2, be2):
    f32 = np.float32
    x = np.asarray(x, f32)
    mask = np.asarray(mask, f32)

    Wv = np.asarray(Wv, f32)
    bv = np.asarray(bv, f32)
    wva = np.zeros((D, VA), f32)
    bva = np.zeros((1, VA), f32)
    for h in range(H):
        wva[:, 65 * h:65 * h + 64] = Wv[:, 64 * h:64 * h + 64]
        bva[0, 65 * h:65 * h + 64] = bv[64 * h:64 * h + 64]
        bva[0, 65 * h + 64] = 1.0

    def pp(v, cols):
        return np.ascontiguousarray(np.asarray(v, f32).reshape(cols, 128).T)

    selp = np.zeros((H, 128), f32)
    for h in range(H):
        selp[h, (h % 2) * 64:(h % 2) * 64 + 64] = 1.0

    bf16 = ml_dtypes.bfloat16
    common = {
        "id8": np.eye(H, dtype=f32),
        "selp": selp,
        "wq": np.asarray(Wq, f32).astype(bf16),
        "wk": np.asarray(Wk, f32).astype(bf16),
        "wva": wva.astype(bf16),
        "wo": np.asarray(Wo, f32).astype(bf16),
        "w1": np.asarray(W1, f32).astype(bf16),
        "w2": np.asarray(W2, f32).astype(bf16),
        "bq": pp(bq, KD),
        "bk": pp(bk, KD),
        "bva": np.repeat(bva, 128, axis=0).astype(bf16),
        "bo": pp(bo, KD),
        "b1": pp(b1, KF),
        "b2": pp(b2, KD),
        "g1": pp(g1, KD),
        "be1": pp(be1, KD),
        "g2": pp(g2, KD),
        "be2": pp(be2, KD),
    }
    in_maps = []
    for c in range(N_CORES):
        b, r = c // 2, c % 2
        js = slice(r * SJ, (r + 1) * SJ)
        ps = slice((1 - r) * SJ, (2 - r) * SJ)
        # local token order: own half first (both in h columns and mask rows)
        xb = x[b].T
        m = dict(common)
        xtl = np.ascontiguousarray(np.concatenate([xb[:, js], xb[:, ps]], axis=1))
        m["xT"] = np.ascontiguousarray(xtl[:, 0:SJ])
        m["xb"] = xtl.astype(bf16)
        mrows = np.concatenate([mask[b][js], mask[b][ps]], axis=0)
        m["maskp"] = np.ascontiguousarray(1.0 - mrows[:, js]).astype(bf16)
        in_maps.append(m)
    return in_maps


def get_nc():
    if "nc" not in _CACHE:
        _CACHE["nc"] = build()
    return _CACHE["nc"]


def run(in_maps, **kw):
    nc = get_nc()
    return run_bass_kernel_spmd(nc, in_maps, core_ids=list(range(N_CORES)), **kw)


def kernel(**inputs):
    in_maps = _prep_inputs(**inputs)
    res = run(in_maps)
    out = np.empty((B, S, D), np.float32)
    for c in range(N_CORES):
        b, r = c // 2, c % 2
        out[b, r * SJ:(r + 1) * SJ, :] = res.results[c]["hout"].T
    return out
